# revision 33
# baseline (speedup 1.0000x reference)
"""Trainium2 Bass kernel for nn_CNN2DAttWind_NL (conv + 32-head rank-1 attention + MLP).

Contract: kernel(**inputs) takes FULL unsharded numpy inputs, returns FULL output
[8192, 5] float32.  Internally shards the batch across 8 NeuronCores (pure data
parallel) and runs a Bass/Tile kernel via run_bass_kernel_spmd.

Layout strategy per core (B_local = 1024):
  - conv as matmul over im2col patches: K=(cin,dy,dx)=64, M=128 out-channels
    (32 conv_out + 32 q + 32 k + 32 v), cols=(b, 25 output pixels).
  - attention: partitions = (b mod 4, head) = 128 rows; free = (b', i, j).
    products q_i*k_j on DVE, exp on ACT, *v + two 25-wide segmented reductions
    on DVE, fast reciprocal for the softmax division.
  - 1x1 conv, then 3-layer MLP as PE matmuls (dense1 accumulates over the 25
    pixels in PSUM; biases applied via ACT bias on eviction).
"""

import os
from contextlib import ExitStack

import numpy as np

# Problem constants (hardcoded per harness contract)
B, CIN, HIN, WIN = 8192, 16, 6, 6
FM = 64
DK = FM // 2          # 32
DV = FM // 2          # 32
NH = FM // 2          # 32 heads, dkh = dvh = 1
OUT_CH = 5
HID = 128
HW = 25               # 5x5 output pixels

NCORES = 8
BL = B // NCORES      # 1024 samples per core
CB = 128              # samples per outer chunk
NCHUNK = BL // CB     # 8
BP = CB // 4          # 32 b' groups per chunk (pair partition = (b%4, head))
SB = 8                # b' groups per attention sub-chunk -> 32 samples
NSUB = BP // SB       # 4


def _build_program(debug_dump=False, repeat=1):
    """Build the Bass program once; returns (nc, input_names, output_name)."""
    import concourse.bass as bass
    import concourse.tile as tile
    from concourse import bacc, mybir

    f32 = mybir.dt.float32
    f16 = mybir.dt.float16
    AF = mybir.ActivationFunctionType
    ALU = mybir.AluOpType
    AX = mybir.AxisListType

    nc = bacc.Bacc(
        "TRN2",
        target_bir_lowering=False,
        debug=False,
        enable_asserts=False,
        num_devices=NCORES,
    )

    # ---- DRAM I/O ----
    x_d = nc.dram_tensor("x_s", [BL, CIN, HIN, WIN], f32, kind="ExternalInput")
    wp32_d = nc.dram_tensor("wp32", [128, 517], f32, kind="ExternalInput")
    wp16_d = nc.dram_tensor("wp16", [128, 3301], f16, kind="ExternalInput")
    out_d = nc.dram_tensor("out", [5, BL], f32, kind="ExternalOutput")
    dbg = {}
    if debug_dump:
        dbg["qkv"] = nc.dram_tensor("dbg_qkv", [96, CB, 25], f16, kind="ExternalOutput")
        dbg["qt"] = nc.dram_tensor("dbg_qt", [128, BP, 25], f16, kind="ExternalOutput")
        dbg["z"] = nc.dram_tensor("dbg_z", [128, SB, 25, 25], f16, kind="ExternalOutput")
        dbg["den"] = nc.dram_tensor("dbg_den", [128, SB, 25], f32, kind="ExternalOutput")
        dbg["att"] = nc.dram_tensor("dbg_att", [128, BP, 25], f16, kind="ExternalOutput")
        dbg["yt"] = nc.dram_tensor("dbg_yt", [64, CB, 25], f16, kind="ExternalOutput")
        dbg["y1"] = nc.dram_tensor("dbg_y1", [128, CB], f16, kind="ExternalOutput")

    in_names = [t.name for t in (x_d, wp32_d, wp16_d)]

    with tile.TileContext(nc) as tc, ExitStack() as ctx:
        singles = ctx.enter_context(tc.tile_pool(name="singles", bufs=1))
        imc_p = ctx.enter_context(tc.tile_pool(name="imc", bufs=2))
        qkv_p = ctx.enter_context(tc.tile_pool(name="qkv", bufs=2))
        yt_p = ctx.enter_context(tc.tile_pool(name="yt", bufs=2))
        qt_p = ctx.enter_context(tc.tile_pool(name="qt", bufs=2))
        att_p = ctx.enter_context(tc.tile_pool(name="att", bufs=2))
        z_p = ctx.enter_context(tc.tile_pool(name="z", bufs=2))
        e_p = ctx.enter_context(tc.tile_pool(name="e", bufs=2))
        ev_p = ctx.enter_context(tc.tile_pool(name="ev", bufs=2))
        red_p = ctx.enter_context(tc.tile_pool(name="red", bufs=3))
        mlp_p = ctx.enter_context(tc.tile_pool(name="mlp", bufs=2))
        outp_p = ctx.enter_context(tc.tile_pool(name="outp", bufs=2))
        ps_conv = ctx.enter_context(tc.tile_pool(name="psc", bufs=2, space="PSUM"))
        ps_att = ctx.enter_context(tc.tile_pool(name="psa", bufs=2, space="PSUM"))
        ps_mlp = ctx.enter_context(tc.tile_pool(name="psm", bufs=1, space="PSUM"))

        # ---- load all weights in 2 DMAs, then barrier ----
        wp32 = singles.tile([128, 517], f32)
        nc.sync.dma_start(out=wp32, in_=wp32_d.ap())
        wp16 = singles.tile([128, 3301], f16)
        nc.sync.dma_start(out=wp16, in_=wp16_d.ap())
        cb_s = wp32[:, 0:1]
        expb_s = wp32[:, 1:2]
        b1_s = wp32[:, 2:3]
        b2_s = wp32[0:64, 3:4]
        ab_s = wp32[0:32, 4:5]
        wconv_s = wp32[0:16, 5:517].rearrange("p (g c) -> p g c", g=4)
        w2_s = wp16[:, 0:64]
        w1_s = wp16[0:64, 64:3264].rearrange("p (h i) -> p h i", i=25)
        aw_s = wp16[0:32, 3264:3296]
        w3_s = wp16[0:64, 3296:3301]

        for c in range(NCHUNK * repeat):
            c = c % NCHUNK
            b0 = c * CB
            # ---- x load (contiguous): xa[cin, b, 36] ----
            xa = imc_p.tile([16, CB, 36], f32)
            src = bass.AP(tensor=x_d, offset=b0 * 576,
                          ap=[[36, 16], [576, CB], [1, 36]])
            nc.sync.dma_start(out=xa, in_=src)

            # ---- conv as 4 accumulating K=16 matmuls (one per 2x2 tap) ----
            qkv_s = qkv_p.tile([96, CB, 25], f16)   # rows: q(0:32) k(32:64) v(64:96)
            y_t = yt_p.tile([64, CB, 25], f16)      # rows 0:32 relu(conv_out), 32:64 attn out
            NB = 512 // 25                          # 20 samples per col-tile
            for s0 in range(0, CB, NB):
                sb_n = min(NB, CB - s0)
                tn = sb_n * 25
                pscv = ps_conv.tile([128, 512], f32)
                for dydx in range(4):
                    dy, dx = dydx // 2, dydx % 2
                    rhs = bass.AP(
                        tensor=xa.tensor,
                        offset=xa.offset + s0 * 36 + dy * 6 + dx,
                        ap=[xa.ap[0], [36, sb_n], [6, 5], [1, 5]],
                    )
                    nc.tensor.matmul(pscv[:, :tn], wconv_s[:, dydx], rhs,
                                     start=(dydx == 0), stop=(dydx == 3))
                # conv_out rows: relu(x + bias) on ACT
                nc.scalar.activation(
                    y_t[0:32, s0:s0 + sb_n],
                    pscv[0:32, :tn].rearrange("p (b i) -> p b i", i=25),
                    AF.Relu, bias=cb_s[0:32], scale=1.0)
                # qkv rows: add bias on DVE (cast to f16), 32 partitions per call
                for g in range(3):
                    p0 = 32 + 32 * g
                    nc.vector.tensor_scalar_add(
                        qkv_s[32 * g:32 * g + 32, s0:s0 + sb_n],
                        pscv[p0:p0 + 32, :tn].rearrange("p (b i) -> p b i", i=25),
                        cb_s[p0:p0 + 32])

            # ---- shuffle qkv -> pair-major layout [ (b4, n), b', i ] ----
            q_t = qt_p.tile([128, BP, 25], f16, tag="q_t")
            q_d = qt_p.tile([128, BP, 25, 2], f16, tag="q_d")   # duplicated pairs
            k_t = qt_p.tile([128, BP, 26], f16, tag="k_t")      # 26-padded
            v_t = qt_p.tile([128, BP, 26], f16, tag="v_t")
            qkv_v = qkv_s.rearrange("p (b2 f) i -> p f b2 i", f=4)
            for b4 in range(4):
                nc.sync.dma_start(out=q_t[32 * b4:32 * b4 + 32], in_=qkv_v[0:32, b4])
                nc.sync.dma_start(out=k_t[32 * b4:32 * b4 + 32, :, 0:25],
                                  in_=qkv_v[32:64, b4])
                nc.sync.dma_start(out=v_t[32 * b4:32 * b4 + 32, :, 0:25],
                                  in_=qkv_v[64:96, b4])
            nc.vector.memset(k_t[:, :, 25:26], 0.0)
            nc.scalar.copy(q_d[:, :, :, 0:1], q_t.unsqueeze(3))
            nc.scalar.copy(q_d[:, :, :, 1:2], q_t.unsqueeze(3))

            # ---- attention ----
            attn_t = att_p.tile([128, BP, 25], f16, tag="attn_t")
            for s in range(NSUB):
                sl = slice(SB * s, SB * (s + 1))
                z = z_p.tile([128, SB, 25, 26], f16)
                q_bc = bass.AP(tensor=q_d.tensor, offset=q_d.offset + SB * s * 50,
                               ap=[q_d.ap[0], [50, SB], [2, 25], [0, 13], [1, 2]])
                k_bc = bass.AP(tensor=k_t.tensor, offset=k_t.offset + SB * s * 26,
                               ap=[k_t.ap[0], [26, SB], [0, 25], [1, 26]])
                v_bc = bass.AP(tensor=v_t.tensor, offset=v_t.offset + SB * s * 26,
                               ap=[v_t.ap[0], [26, SB], [0, 25], [1, 25]])
                nc.vector.tensor_mul(z, q_bc, k_bc)
                E = e_p.tile([128, SB, 25, 26], f16)
                nc.scalar.activation(E, z, AF.Exp, bias=expb_s, scale=1.0)
                Ev = ev_p.tile([128, SB, 25, 25], f16)
                nc.vector.tensor_mul(Ev, E[:, :, :, 0:25], v_bc)
                num = red_p.tile([128, SB, 25], f32, tag="num")
                den = red_p.tile([128, SB, 25], f32, tag="den")
                nc.vector.tensor_reduce(num, Ev, axis=AX.X, op=ALU.add)
                nc.vector.tensor_reduce(den, E[:, :, :, 0:25], axis=AX.X, op=ALU.add)
                rden = red_p.tile([128, SB, 25], f32, tag="rden")
                nc.vector.reciprocal_approx_fast(rden, den)
                nc.vector.tensor_mul(attn_t[:, sl, :], num, rden)
                if debug_dump and c == 0 and s == 0:
                    nc.sync.dma_start(out=dbg["z"].ap(), in_=z[:, :, :, 0:25])
                    nc.sync.dma_start(out=dbg["den"].ap(), in_=den)

            # ---- shuffle attention back to [head, (b, i)] ----
            attn_s = att_p.tile([32, CB, 25], f16, tag="attn_s")
            attn_sv = attn_s.rearrange("p (b2 f) i -> p f b2 i", f=4)
            for b4 in range(4):
                nc.sync.dma_start(out=attn_sv[:, b4],
                                  in_=attn_t[32 * b4:32 * b4 + 32])

            # ---- 1x1 conv + relu -> y_t rows 32:64 ----
            NCOLS = CB * 25
            atf = attn_s.rearrange("p b i -> p (b i)")
            for t0 in range(0, NCOLS, 512):
                tn = min(512, NCOLS - t0)
                psat = ps_att.tile([32, 512], f32)
                nc.tensor.matmul(psat[:, :tn], aw_s, atf[:, t0:t0 + tn],
                                 start=True, stop=True)
                nc.scalar.activation(
                    y_t.rearrange("p b i -> p (b i)")[32:64, t0:t0 + tn],
                    psat[:, :tn], AF.Relu, bias=ab_s, scale=1.0)

            # ---- dense1: accumulate over 25 pixels ----
            ps1 = ps_mlp.tile([128, CB], f32, tag="ps1")
            for i in range(25):
                nc.tensor.matmul(ps1, w1_s[:, :, i], y_t[:, :, i],
                                 start=(i == 0), stop=(i == 24))
            y1 = mlp_p.tile([128, CB], f16, tag="y1")
            nc.scalar.activation(y1, ps1, AF.Relu, bias=b1_s, scale=1.0)

            # ---- dense2 ----
            ps2 = ps_mlp.tile([64, CB], f32, tag="ps2")
            nc.tensor.matmul(ps2, w2_s, y1, start=True, stop=True)
            y2 = mlp_p.tile([64, CB], f16, tag="y2")
            nc.scalar.activation(y2, ps2, AF.Relu, bias=b2_s, scale=1.0)

            if debug_dump and c == 0:
                nc.sync.dma_start(out=dbg["qkv"].ap(), in_=qkv_s)
                nc.sync.dma_start(out=dbg["qt"].ap(), in_=q_t)
                nc.sync.dma_start(out=dbg["att"].ap(), in_=attn_t)
                nc.sync.dma_start(out=dbg["yt"].ap(), in_=y_t)
                nc.sync.dma_start(out=dbg["y1"].ap(), in_=y1)

            # ---- dense3 (bias added on host) ----
            ps3 = ps_mlp.tile([5, CB], f32, tag="ps3")
            nc.tensor.matmul(ps3, w3_s, y2, start=True, stop=True)
            outs = outp_p.tile([5, CB], f32)
            nc.scalar.copy(outs, ps3)
            nc.sync.dma_start(
                out=bass.AP(tensor=out_d, offset=b0, ap=[[BL, 5], [1, CB]]),
                in_=outs)

    nc.finalize()
    return nc, in_names, out_d.name


_PROG = None


def _get_program():
    global _PROG
    if _PROG is None:
        _PROG = _build_program()
    return _PROG


def _host_conv(x, w, b):
    """2x2 VALID conv, NCHW, numpy. Returns [B, O, 25] float32."""
    B_, C_, H_, W_ = x.shape
    out = None
    for dy in range(2):
        for dx in range(2):
            xs = x[:, :, dy:dy + 5, dx:dx + 5].reshape(B_, C_, 25)
            t = np.einsum('oc,bcp->bop', w[:, :, dy, dx], xs,
                          optimize=True)
            out = t if out is None else out + t
    return (out + b[None, :, None]).astype(np.float32)


def _make_in_maps(inputs):
    return _host_prep(**inputs)


def _host_prep(x, conv_w, conv_b, qkv_w, qkv_b, attn_w, attn_b,
               w1, b1, w2, b2, w3, b3):
    # host-side weight prep
    wc = np.concatenate([np.asarray(conv_w), np.asarray(qkv_w)], axis=0)  # [128,16,2,2]
    # wconv[cin, (2dy+dx), ch] for the 4 accumulating K=16 conv matmuls
    wconv = np.ascontiguousarray(
        wc.transpose(1, 2, 3, 0).reshape(16, 4, 128)).astype(np.float32)
    cbias = np.concatenate([np.asarray(conv_b), np.asarray(qkv_b)])[:, None].astype(np.float32)
    aw = np.ascontiguousarray(np.asarray(attn_w)[:, :, 0, 0].T).astype(np.float16)
    ab = np.asarray(attn_b)[:, None].astype(np.float32)
    w1t = np.ascontiguousarray(
        np.asarray(w1).reshape(HID, 64, 25).transpose(1, 0, 2)).astype(np.float16)
    b1c = np.asarray(b1)[:, None].astype(np.float32)
    w2t = np.ascontiguousarray(np.asarray(w2).T).astype(np.float16)
    b2c = np.asarray(b2)[:, None].astype(np.float32)
    w3t = np.ascontiguousarray(np.asarray(w3).T).astype(np.float16)

    x = np.asarray(x, dtype=np.float32)

    # exp-overflow guard: softmax is invariant to exp(z - C0); pick C0 from the
    # exact global max of q_i*k_j (corner products of per-row min/max).
    qw, kw = np.asarray(qkv_w)[0:32], np.asarray(qkv_w)[32:64]
    qb_, kb_ = np.asarray(qkv_b)[0:32], np.asarray(qkv_b)[32:64]
    qv = _host_conv(x, qw, qb_)        # [B, 32, 25]
    kv = _host_conv(x, kw, kb_)
    qmax, qmin = qv.max(2), qv.min(2)  # [B, 32]
    kmax, kmin = kv.max(2), kv.min(2)
    zmax = max((qmax * kmax).max(), (qmax * kmin).max(),
               (qmin * kmax).max(), (qmin * kmin).max())
    c0 = float(max(0.0, zmax - 8.8))
    expb = np.full((128, 1), -c0, dtype=np.float32)

    wp32 = np.zeros((128, 517), np.float32)
    wp32[:, 0:1] = cbias
    wp32[:, 1:2] = expb
    wp32[:, 2:3] = b1c
    wp32[0:64, 3:4] = b2c
    wp32[0:32, 4:5] = ab
    wp32[0:16, 5:517] = wconv.reshape(16, 512)
    wp16 = np.zeros((128, 3301), np.float16)
    wp16[:, 0:64] = w2t
    wp16[0:64, 64:3264] = w1t.reshape(64, 3200)
    wp16[0:32, 3264:3296] = aw
    wp16[0:64, 3296:3301] = w3t

    shared = {"wp32": wp32, "wp16": wp16}
    in_maps = []
    for c in range(NCORES):
        m = dict(shared)
        m["x_s"] = np.ascontiguousarray(x[c * BL:(c + 1) * BL])
        in_maps.append(m)
    return in_maps


def kernel(x, conv_w, conv_b, qkv_w, qkv_b, attn_w, attn_b,
           w1, b1, w2, b2, w3, b3):
    from concourse.bass_utils import run_bass_kernel_spmd

    nc, in_names, out_name = _get_program()
    in_maps = _host_prep(x, conv_w, conv_b, qkv_w, qkv_b, attn_w, attn_b,
                         w1, b1, w2, b2, w3, b3)
    res = run_bass_kernel_spmd(nc, in_maps, core_ids=list(range(NCORES)))
    outs = [r[out_name] for r in res.results]           # each [5, BL]
    full = np.concatenate([o.T for o in outs], axis=0)  # [8192, 5]
    full = full + np.asarray(b3)[None, :].astype(np.float32)
    return full.astype(np.float32)
